# revision 24
# baseline (speedup 1.0000x reference)
"""Segment-mean + projection kernel for Trainium2 (8 NeuronCores, SPMD).

logits[b] = (mean of x rows in bag b) @ rel_weight.T + bias

Strategy: data-parallel over bags. Each core gets a bag-aligned slice of
rows, padded to G groups of 768 rows (6 tiles of 128). Per 128-row tile the
DVE builds a one-hot matrix A[p, f] = (seg_local[p] == f) and the PE
accumulates A.T @ x into PSUM over the group's 6 tiles (fp32r matmuls).
Bags split across a group boundary are repaired with a rank-1 fixup matmul
(one-hot row DMA'd from host). Means = PSUM * (1/count) per column, then
PE-transposed into [D, bags] layout and projected against W.T chunks, bias
added, emitted as logitsT [53, bags-slots]; the host compacts the valid
columns. All data-dependent structure travels as DMA'd tensors, so one
program serves all 8 cores.
"""
import sys
import re

sys.path.insert(0, "/opt/trn_rl_repo")

import numpy as np

N_CORES = 8
SERIAL_BUFS = 0  # set 1 to serialize pipeline for debug
ROWS_PER_TILE = 128
TILES_PER_GROUP = 6
ROWS_PER_GROUP = ROWS_PER_TILE * TILES_PER_GROUP  # 768
D = 690
D_SPLIT = 344  # fp32r moving dim must be even; 344 + 346
C = 53
D_CHUNKS = 6  # ceil(690 / 128); last chunk is 50 wide
D_LAST = D - 5 * 128  # 50


def _apply_walrus_workarounds():
    """This walrus build allows at most one semaphore wait per instruction
    on several opcodes (Drain, Matmult/LDW). Patch Tile's tail drain to use
    standalone wait_ge instructions, and provide a post-pass that hoists
    excess waits onto InstNoOp instructions."""
    from concourse import tile, mybir

    def _patched_drain_and_barrier(self, tick_clock, wait_clock):
        gc = tick_clock.global_clock
        ticks = [int(s) for s in re.findall(r"\d+", repr(gc))]
        allocated = self.sems.allocated()
        for proc, sem in sorted(allocated.items()):
            t = ticks[proc] if proc < len(ticks) else 0
            if t > 0:
                mult = 16 if "DMA" in sem.name else 1
                self.nc.sync.wait_ge(sem, t * mult)
        self.nc.sync.drain()
        self.nc.all_engine_barrier()
        popped = self.nc._tile_sem_poison_stack.pop()
        assert popped is self._sem_poison
        self.nc.clear_and_free_semaphores(list(allocated.values()))
        self.nc.all_engine_barrier()

    tile.TileContext._drain_and_barrier = _patched_drain_and_barrier

    def split_multi_waits(nc, max_waits=1):
        for f in nc.m.functions:
            for b in f.blocks:
                insts = list(b.instructions)
                new = []
                dirty = False
                for inst in insts:
                    si = inst.sync_info
                    if si is not None and len(si.on_wait) > max_waits:
                        waits = list(si.on_wait)
                        extra, keep = waits[:-max_waits], waits[-max_waits:]
                        for k, w in enumerate(extra):
                            nop = mybir.InstNoOp(
                                name=f"{inst.name}-hw{k}", ins=[], outs=[]
                            )
                            nop.engine = inst.engine
                            nop.sync_info = mybir.SyncInfo(
                                on_wait=[w], on_update=[]
                            )
                            new.append(nop)
                        inst.sync_info = mybir.SyncInfo(
                            on_wait=keep, on_update=list(si.on_update)
                        )
                        dirty = True
                    new.append(inst)
                if dirty:
                    b.instructions = new

    return split_multi_waits


def _preprocess(x, scope, n_cores=N_CORES):
    """Compute per-core padded row slices and all data-dependent side
    tensors for the SPMD program."""
    n_sent = x.shape[0]
    n_bags = scope.shape[0] - 1
    scope = np.asarray(scope, dtype=np.int64)
    counts = np.diff(scope)
    assert counts.min() >= 1
    assert counts.max() < ROWS_PER_GROUP, "a bag may span at most 2 groups"
    seg_full = np.repeat(np.arange(n_bags, dtype=np.int64), counts)

    # bag-aligned row cuts near k * n_sent / n_cores
    row_cuts = [0]
    bag_cuts = [0]
    for k in range(1, n_cores):
        t = (k * n_sent) // n_cores
        b = int(np.searchsorted(scope, t, side="right")) - 1
        bag_cuts.append(b)
        row_cuts.append(int(scope[b]))
    row_cuts.append(n_sent)
    bag_cuts.append(n_bags)

    rows_per_core = [row_cuts[c + 1] - row_cuts[c] for c in range(n_cores)]
    G = int(np.ceil(max(rows_per_core) / ROWS_PER_GROUP))
    R = G * ROWS_PER_GROUP
    n_pairs = (G + 1) // 2

    cores = []
    for c in range(n_cores):
        r0, r1 = row_cuts[c], row_cuts[c + 1]
        b0, b1 = bag_cuts[c], bag_cuts[c + 1]
        nrows = r1 - r0

        x_pad = np.zeros((R, D), dtype=np.float16)
        x_pad[:nrows] = x[r0:r1].astype(np.float16)

        seg_c = seg_full[r0:r1]  # global bag ids
        # base bag per group; B1 sentinel for pad groups
        base = np.empty(G + 1, dtype=np.int64)
        for g in range(G + 1):
            rr = g * ROWS_PER_GROUP
            base[g] = seg_c[rr] if rr < nrows else b1

        seg_local = np.full(R, 128.0, dtype=np.float32)
        grp = np.arange(nrows) // ROWS_PER_GROUP
        seg_local[:nrows] = (seg_c - base[grp]).astype(np.float32)
        assert seg_local[:nrows].max(initial=0.0) <= 127.0

        fixup = np.zeros((G, 128), dtype=np.float32)
        start_col = np.zeros(G, dtype=np.int64)
        end_col = np.full(G, -1, dtype=np.int64)
        nxt_start = 0  # start col of group g computed from g-1's overlap
        for g in range(G):
            rr_end = (g + 1) * ROWS_PER_GROUP
            nb = base[g + 1]
            if g * ROWS_PER_GROUP >= nrows:
                # pad group: owns nothing
                start_col[g], end_col[g] = 1, 0
                continue
            start_col[g] = nxt_start
            if rr_end < nrows and int(scope[nb]) - r0 < rr_end:
                # bag nb has rows in both g and g+1: g owns it, fixup adds
                # g+1's partial (always at S_{g+1}[0])
                L = int(nb - base[g])
                assert 1 <= L <= 127
                fixup[g, L] = 1.0
                end_col[g] = L
                nxt_start = 1
            else:
                end_col[g] = int(nb - 1 - base[g])
                nxt_start = 0

        # recip counts per group column
        recip = np.ones((G, 128), dtype=np.float32)
        for g in range(G):
            lo = base[g]
            hi = min(lo + 128, b1)
            if hi > lo:
                recip[g, : hi - lo] = 1.0 / counts[lo:hi]

        # seg_local as [128, G*6] (col = g*6+t), recip as [128, G]
        seg_sb = seg_local.reshape(G * TILES_PER_GROUP, 128).T.copy()
        recip_sb = recip.T.copy()  # [128, G]

        cores.append(
            dict(
                x=x_pad,
                seg=np.ascontiguousarray(seg_sb),
                recip=np.ascontiguousarray(recip_sb),
                fixup=fixup.reshape(1, G * 128).copy(),
                base=base,
                start_col=start_col,
                end_col=end_col,
                b0=b0,
                b1=b1,
            )
        )
    return cores, G, n_pairs


def _build_program(G, n_pairs, rel_weight, bias):
    import concourse.bass as bass
    import concourse.mybir as mybir
    from concourse import tile

    dt = mybir.dt
    nc = bass.Bass()

    x_d = nc.declare_dram_parameter(
        "x", [G * ROWS_PER_GROUP, D], dt.float16, isOutput=False
    )
    seg_d = nc.declare_dram_parameter(
        "seg", [128, G * TILES_PER_GROUP], dt.float32, isOutput=False
    )
    recip_d = nc.declare_dram_parameter(
        "recip", [128, G], dt.float32, isOutput=False
    )
    fixup_d = nc.declare_dram_parameter(
        "fixup", [1, G * 128], dt.float16, isOutput=False
    )
    iota_d = nc.declare_dram_parameter("iota", [128, 128], dt.float32, isOutput=False)
    ident_d = nc.declare_dram_parameter(
        "ident", [128, 128], dt.float32, isOutput=False
    )
    wt_d = nc.declare_dram_parameter("wt", [128, 768], dt.float32, isOutput=False)
    bias_d = nc.declare_dram_parameter("bias", [C, 1], dt.float32, isOutput=False)
    out_d = nc.declare_dram_parameter(
        "out", [C, n_pairs * 256], dt.float32, isOutput=True
    )

    with tile.TileContext(nc) as tc:
        with (
            tc.tile_pool(name="const", bufs=1) as cpool,
            tc.tile_pool(name="xin", bufs=SERIAL_BUFS or 4) as xpool,
            tc.tile_pool(name="onehot", bufs=SERIAL_BUFS or 6) as apool,
            tc.tile_pool(name="rows", bufs=SERIAL_BUFS or 2) as rpool,
            tc.tile_pool(name="means", bufs=SERIAL_BUFS or 2) as mpool,
            tc.tile_pool(name="mgt", bufs=SERIAL_BUFS or 2) as tpool,
            tc.tile_pool(name="outs", bufs=2) as opool,
            tc.tile_pool(name="ps_sum", bufs=2, space="PSUM") as pspool,
            tc.tile_pool(name="ps_tr", bufs=2, space="PSUM") as ptpool,
            tc.tile_pool(name="ps_proj", bufs=2, space="PSUM") as pppool,
        ):
            iota_t = cpool.tile([128, 128], dt.float32)
            ident_t = cpool.tile([128, 128], dt.float32)
            seg_t = cpool.tile([128, G * TILES_PER_GROUP], dt.float32)
            recip_t = cpool.tile([128, G], dt.float32)
            fixup_t = cpool.tile([1, G * 128], dt.float16)
            wt_t = cpool.tile([128, 768], dt.float32)
            wtr_t = cpool.tile([128, 768], dt.float32r)
            bias_t = cpool.tile([C, 1], dt.float32)

            nc.gpsimd.dma_start(out=iota_t[:], in_=iota_d[:])
            nc.gpsimd.dma_start(out=ident_t[:], in_=ident_d[:])
            nc.gpsimd.dma_start(out=seg_t[:], in_=seg_d[:])
            nc.gpsimd.dma_start(out=recip_t[:], in_=recip_d[:])
            nc.gpsimd.dma_start(out=fixup_t[:], in_=fixup_d[:])
            nc.gpsimd.dma_start(out=wt_t[:], in_=wt_d[:])
            nc.vector.tensor_copy(wtr_t[:], wt_t[:])
            nc.gpsimd.dma_start(out=bias_t[:], in_=bias_d[:])



            prev = None  # (ps_a, ps_b, g-1)
            r_tile = None
            mgt = None

            for g in range(G + 1):
                cur = None
                if g < G:
                    x_t = xpool.tile(
                        [128, TILES_PER_GROUP * D], dt.float16, tag="x"
                    )
                    nc.sync.dma_start(
                        out=x_t[:].rearrange("p (t d) -> p t d", d=D),
                        in_=x_d[
                            g * ROWS_PER_GROUP : (g + 1) * ROWS_PER_GROUP, :
                        ].rearrange("(t p) d -> p t d", p=ROWS_PER_TILE),
                    )
                    ps_a = pspool.tile([128, D_SPLIT], dt.float32, tag="psa")
                    ps_b = pspool.tile([128, D - D_SPLIT], dt.float32, tag="psb")
                    for t in range(TILES_PER_GROUP):
                        a_t = apool.tile([128, 128], dt.float16, tag="a")
                        col = g * TILES_PER_GROUP + t
                        nc.vector.tensor_scalar(
                            out=a_t[:],
                            in0=iota_t[:],
                            scalar1=seg_t[:, col : col + 1],
                            scalar2=None,
                            op0=mybir.AluOpType.is_equal,
                        )
                        first = t == 0
                        nc.tensor.matmul(
                            ps_a[:],
                            a_t[:],
                            x_t[:, t * D : t * D + D_SPLIT],
                            start=first,
                            stop=False,
                        )
                        nc.tensor.matmul(
                            ps_b[:],
                            a_t[:],
                            x_t[:, t * D + D_SPLIT : (t + 1) * D],
                            start=first,
                            stop=False,
                        )
                    cur = (ps_a, ps_b)
                    if g >= 1:
                        # row 0 of this group's partial sums, for the fixup
                        # of group g-1 (bag split across the boundary)
                        r_tile = rpool.tile([1, D], dt.float16, tag="r")
                        nc.scalar.copy(r_tile[:, 0:D_SPLIT], ps_a[0:1, :])
                        nc.scalar.copy(r_tile[:, D_SPLIT:D], ps_b[0:1, :])

                if g >= 1:
                    pg = g - 1
                    pa, pb = prev
                    # fixup: S_{g-1}[L] += S_g[0]; zero one-hot for no-op.
                    # For g == G reuse the last r_tile (one-hot is zero).
                    fx = fixup_t[:, pg * 128 : (pg + 1) * 128]
                    nc.tensor.matmul(
                        pa[:], fx, r_tile[:, 0:D_SPLIT], start=False, stop=True
                    )
                    nc.tensor.matmul(
                        pb[:], fx, r_tile[:, D_SPLIT:D], start=False, stop=True
                    )
                    # means = psum * recip (per output column of group pg)
                    means = mpool.tile([128, D], dt.float32, tag="m")
                    nc.scalar.activation(
                        means[:, 0:D_SPLIT],
                        pa[:],
                        mybir.ActivationFunctionType.Copy,
                        scale=recip_t[:, pg : pg + 1],
                    )
                    nc.scalar.activation(
                        means[:, D_SPLIT:D],
                        pb[:],
                        mybir.ActivationFunctionType.Copy,
                        scale=recip_t[:, pg : pg + 1],
                    )
                    # wait: recip scaling must be per *partition* = bag row
                    # of the psum ([bag, D] layout) -> scalar1 is [128,1] ok

                    h = pg % 2
                    if h == 0:
                        mgt = tpool.tile([128, 6 * 256], dt.float32r, tag="mgt")
                    for d in range(D_CHUNKS):
                        w = 128 if d < 5 else D_LAST
                        ps_t = ptpool.tile([128, 128], dt.float32, tag="pt")
                        nc.tensor.transpose(
                            ps_t[0:w, :],
                            means[:, d * 128 : d * 128 + w],
                            ident_t[:],
                        )
                        if d % 2 == 0:
                            nc.vector.tensor_copy(
                                mgt[0:w, d * 256 + h * 128 : d * 256 + h * 128 + 128],
                                ps_t[0:w, :],
                            )
                        else:
                            nc.scalar.copy(
                                mgt[0:w, d * 256 + h * 128 : d * 256 + h * 128 + 128],
                                ps_t[0:w, :],
                            )
                    if h == 1 or g == G:
                        q = pg // 2
                        pp = pppool.tile([128, 256], dt.float32, tag="pp")
                        for d in range(D_CHUNKS):
                            w = 128 if d < 5 else D_LAST
                            nc.tensor.matmul(
                                pp[:],
                                wtr_t[0:w, d * 128 : (d + 1) * 128],
                                mgt[0:w, d * 256 : (d + 1) * 256],
                                start=(d == 0),
                                stop=(d == D_CHUNKS - 1),
                            )
                        out_sb = opool.tile([C, 256], dt.float32, tag="o")
                        nc.scalar.activation(
                            out_sb[:],
                            pp[0:C, :],
                            mybir.ActivationFunctionType.Identity,
                            bias=bias_t[:],
                        )
                        nc.gpsimd.dma_start(
                            out=out_d[:, q * 256 : (q + 1) * 256], in_=out_sb[:]
                        )
                prev = cur
    return nc


def prepare(x, scope, rel_weight, bias):
    """Build the SPMD program + per-core input maps. Returns a dict with
    everything needed to execute and assemble the output."""
    split_multi_waits = _apply_walrus_workarounds()

    x = np.asarray(x, dtype=np.float32)
    scope_np = np.asarray(scope)
    rel_weight = np.asarray(rel_weight, dtype=np.float32)
    bias = np.asarray(bias, dtype=np.float32)
    n_bags = scope_np.shape[0] - 1

    cores, G, n_pairs = _preprocess(x, scope_np)
    nc = _build_program(G, n_pairs, rel_weight, bias)
    split_multi_waits(nc)

    iota = np.tile(np.arange(128, dtype=np.float32), (128, 1))
    ident = np.eye(128, dtype=np.float32)
    wt = np.zeros((128, 768), dtype=np.float32)
    wpad = np.zeros((C, 768), dtype=np.float32)
    wpad[:, :D] = rel_weight
    for d in range(6):
        wt[:, d * 128 : d * 128 + C] = wpad[:, d * 128 : (d + 1) * 128].T
    bias_in = bias.reshape(C, 1).copy()

    in_maps = []
    for c in range(N_CORES):
        cd = cores[c]
        in_maps.append(
            {
                "x": cd["x"],
                "seg": cd["seg"],
                "recip": cd["recip"],
                "fixup": cd["fixup"].astype(np.float16),
                "iota": iota,
                "ident": ident,
                "wt": wt,
                "bias": bias_in,
            }
        )

    def assemble(results):
        logits_t = np.empty((C, n_bags), dtype=np.float32)
        for c in range(N_CORES):
            out = results[c]["out"]  # [C, n_pairs*256]
            cd = cores[c]
            base, s_col, e_col = cd["base"], cd["start_col"], cd["end_col"]
            for g in range(G):
                s, e = int(s_col[g]), int(e_col[g])
                if e < s:
                    continue
                col0 = 256 * (g // 2) + 128 * (g % 2)
                bag0 = int(base[g])
                logits_t[:, bag0 + s : bag0 + e + 1] = out[
                    :, col0 + s : col0 + e + 1
                ]
        return np.ascontiguousarray(logits_t.T)

    return dict(nc=nc, in_maps=in_maps, assemble=assemble, G=G, n_pairs=n_pairs)


def kernel(x, scope, rel_weight, bias):
    from concourse.bass_utils import run_bass_kernel_spmd

    p = prepare(x, scope, rel_weight, bias)
    res = run_bass_kernel_spmd(p["nc"], p["in_maps"], list(range(N_CORES)))
    return p["assemble"](res.results)


# revision 26
# speedup vs baseline: 1.1690x; 1.1690x over previous
"""Segment-mean + projection kernel for Trainium2 (8 NeuronCores, SPMD).

logits[b] = (mean of x rows in bag b) @ rel_weight.T + bias

Strategy: data-parallel over bags. Each core gets a bag-aligned slice of
rows, padded to G groups of 768 rows (6 tiles of 128). Per 128-row tile the
DVE builds a one-hot matrix A[p, f] = (seg_local[p] == f) and the PE
accumulates A.T @ x into PSUM over the group's 6 tiles (fp32r matmuls).
Bags split across a group boundary are repaired with a rank-1 fixup matmul
(one-hot row DMA'd from host). Means = PSUM * (1/count) per column, then
PE-transposed into [D, bags] layout and projected against W.T chunks, bias
added, emitted as logitsT [53, bags-slots]; the host compacts the valid
columns. All data-dependent structure travels as DMA'd tensors, so one
program serves all 8 cores.
"""
import sys
import re

sys.path.insert(0, "/opt/trn_rl_repo")

import numpy as np

N_CORES = 8
SERIAL_BUFS = 0  # set 1 to serialize pipeline for debug
ROWS_PER_TILE = 128
TILES_PER_GROUP = 6
ROWS_PER_GROUP = ROWS_PER_TILE * TILES_PER_GROUP  # 768
D = 690
D_SPLIT = 344  # fp32r moving dim must be even; 344 + 346
C = 53
D_CHUNKS = 6  # ceil(690 / 128); last chunk is 50 wide
D_LAST = D - 5 * 128  # 50


def _apply_walrus_workarounds():
    """This walrus build allows at most one semaphore wait per instruction
    on several opcodes (Drain, Matmult/LDW). Patch Tile's tail drain to use
    standalone wait_ge instructions, and provide a post-pass that hoists
    excess waits onto InstNoOp instructions."""
    from concourse import tile, mybir

    def _patched_drain_and_barrier(self, tick_clock, wait_clock):
        gc = tick_clock.global_clock
        ticks = [int(s) for s in re.findall(r"\d+", repr(gc))]
        allocated = self.sems.allocated()
        for proc, sem in sorted(allocated.items()):
            t = ticks[proc] if proc < len(ticks) else 0
            if t > 0:
                mult = 16 if "DMA" in sem.name else 1
                self.nc.sync.wait_ge(sem, t * mult)
        self.nc.sync.drain()
        self.nc.all_engine_barrier()
        popped = self.nc._tile_sem_poison_stack.pop()
        assert popped is self._sem_poison
        self.nc.clear_and_free_semaphores(list(allocated.values()))
        self.nc.all_engine_barrier()

    tile.TileContext._drain_and_barrier = _patched_drain_and_barrier

    def split_multi_waits(nc, max_waits=1):
        for f in nc.m.functions:
            for b in f.blocks:
                insts = list(b.instructions)
                new = []
                dirty = False
                for inst in insts:
                    si = inst.sync_info
                    if si is not None and len(si.on_wait) > max_waits:
                        waits = list(si.on_wait)
                        extra, keep = waits[:-max_waits], waits[-max_waits:]
                        for k, w in enumerate(extra):
                            nop = mybir.InstNoOp(
                                name=f"{inst.name}-hw{k}", ins=[], outs=[]
                            )
                            nop.engine = inst.engine
                            nop.sync_info = mybir.SyncInfo(
                                on_wait=[w], on_update=[]
                            )
                            new.append(nop)
                        inst.sync_info = mybir.SyncInfo(
                            on_wait=keep, on_update=list(si.on_update)
                        )
                        dirty = True
                    new.append(inst)
                if dirty:
                    b.instructions = new

    return split_multi_waits


def _preprocess(x, scope, n_cores=N_CORES):
    """Compute per-core padded row slices and all data-dependent side
    tensors for the SPMD program."""
    n_sent = x.shape[0]
    n_bags = scope.shape[0] - 1
    scope = np.asarray(scope, dtype=np.int64)
    counts = np.diff(scope)
    assert counts.min() >= 1
    assert counts.max() < ROWS_PER_GROUP, "a bag may span at most 2 groups"
    seg_full = np.repeat(np.arange(n_bags, dtype=np.int64), counts)

    # bag-aligned row cuts near k * n_sent / n_cores
    row_cuts = [0]
    bag_cuts = [0]
    for k in range(1, n_cores):
        t = (k * n_sent) // n_cores
        b = int(np.searchsorted(scope, t, side="right")) - 1
        bag_cuts.append(b)
        row_cuts.append(int(scope[b]))
    row_cuts.append(n_sent)
    bag_cuts.append(n_bags)

    rows_per_core = [row_cuts[c + 1] - row_cuts[c] for c in range(n_cores)]
    G = int(np.ceil(max(rows_per_core) / ROWS_PER_GROUP))
    R = G * ROWS_PER_GROUP
    n_pairs = (G + 1) // 2

    cores = []
    for c in range(n_cores):
        r0, r1 = row_cuts[c], row_cuts[c + 1]
        b0, b1 = bag_cuts[c], bag_cuts[c + 1]
        nrows = r1 - r0

        x_pad = np.zeros((R, D), dtype=np.float16)
        x_pad[:nrows] = x[r0:r1].astype(np.float16)

        seg_c = seg_full[r0:r1]  # global bag ids
        # base bag per group; B1 sentinel for pad groups
        base = np.empty(G + 1, dtype=np.int64)
        for g in range(G + 1):
            rr = g * ROWS_PER_GROUP
            base[g] = seg_c[rr] if rr < nrows else b1

        seg_local = np.full(R, 128.0, dtype=np.float32)
        grp = np.arange(nrows) // ROWS_PER_GROUP
        seg_local[:nrows] = (seg_c - base[grp]).astype(np.float32)
        assert seg_local[:nrows].max(initial=0.0) <= 127.0

        fixup = np.zeros((G, 128), dtype=np.float32)
        start_col = np.zeros(G, dtype=np.int64)
        end_col = np.full(G, -1, dtype=np.int64)
        nxt_start = 0  # start col of group g computed from g-1's overlap
        for g in range(G):
            rr_end = (g + 1) * ROWS_PER_GROUP
            nb = base[g + 1]
            if g * ROWS_PER_GROUP >= nrows:
                # pad group: owns nothing
                start_col[g], end_col[g] = 1, 0
                continue
            start_col[g] = nxt_start
            if rr_end < nrows and int(scope[nb]) - r0 < rr_end:
                # bag nb has rows in both g and g+1: g owns it, fixup adds
                # g+1's partial (always at S_{g+1}[0])
                L = int(nb - base[g])
                assert 1 <= L <= 127
                fixup[g, L] = 1.0
                end_col[g] = L
                nxt_start = 1
            else:
                end_col[g] = int(nb - 1 - base[g])
                nxt_start = 0

        # recip counts per group column
        recip = np.ones((G, 128), dtype=np.float32)
        for g in range(G):
            lo = base[g]
            hi = min(lo + 128, b1)
            if hi > lo:
                recip[g, : hi - lo] = 1.0 / counts[lo:hi]

        # seg_local as [128, G*6] (col = g*6+t), recip as [128, G]
        seg_sb = seg_local.reshape(G * TILES_PER_GROUP, 128).T.copy()
        recip_sb = recip.T.copy()  # [128, G]

        cores.append(
            dict(
                x=x_pad,
                seg=np.ascontiguousarray(seg_sb),
                recip=np.ascontiguousarray(recip_sb),
                fixup=fixup.reshape(1, G * 128).copy(),
                base=base,
                start_col=start_col,
                end_col=end_col,
                b0=b0,
                b1=b1,
            )
        )
    return cores, G, n_pairs


def _build_program(G, n_pairs, rel_weight, bias):
    import concourse.bass as bass
    import concourse.mybir as mybir
    from concourse import tile

    dt = mybir.dt
    nc = bass.Bass()

    x_d = nc.declare_dram_parameter(
        "x", [G * ROWS_PER_GROUP, D], dt.float16, isOutput=False
    )
    seg_d = nc.declare_dram_parameter(
        "seg", [128, G * TILES_PER_GROUP], dt.float32, isOutput=False
    )
    recip_d = nc.declare_dram_parameter(
        "recip", [128, G], dt.float32, isOutput=False
    )
    fixup_d = nc.declare_dram_parameter(
        "fixup", [1, G * 128], dt.float16, isOutput=False
    )
    iota_d = nc.declare_dram_parameter("iota", [128, 128], dt.float32, isOutput=False)
    ident_d = nc.declare_dram_parameter(
        "ident", [128, 128], dt.float16, isOutput=False
    )
    wt_d = nc.declare_dram_parameter("wt", [128, 768], dt.float16, isOutput=False)
    bias_d = nc.declare_dram_parameter("bias", [C, 1], dt.float32, isOutput=False)
    out_d = nc.declare_dram_parameter(
        "out", [C, n_pairs * 256], dt.float32, isOutput=True
    )

    with tile.TileContext(nc) as tc:
        with (
            tc.tile_pool(name="const", bufs=1) as cpool,
            tc.tile_pool(name="xin", bufs=SERIAL_BUFS or 4) as xpool,
            tc.tile_pool(name="onehot", bufs=SERIAL_BUFS or 6) as apool,
            tc.tile_pool(name="rows", bufs=SERIAL_BUFS or 2) as rpool,
            tc.tile_pool(name="means", bufs=SERIAL_BUFS or 2) as mpool,
            tc.tile_pool(name="mgt", bufs=SERIAL_BUFS or 2) as tpool,
            tc.tile_pool(name="outs", bufs=2) as opool,
            tc.tile_pool(name="ps_sum", bufs=2, space="PSUM") as pspool,
            tc.tile_pool(name="ps_tr", bufs=2, space="PSUM") as ptpool,
            tc.tile_pool(name="ps_proj", bufs=2, space="PSUM") as pppool,
        ):
            iota_t = cpool.tile([128, 128], dt.float32)
            ident_t = cpool.tile([128, 128], dt.float16)
            seg_t = cpool.tile([128, G * TILES_PER_GROUP], dt.float32)
            recip_t = cpool.tile([128, G], dt.float32)
            fixup_t = cpool.tile([1, G * 128], dt.float16)
            wt_t = cpool.tile([128, 768], dt.float16)
            bias_t = cpool.tile([C, 1], dt.float32)

            nc.gpsimd.dma_start(out=iota_t[:], in_=iota_d[:])
            nc.gpsimd.dma_start(out=ident_t[:], in_=ident_d[:])
            nc.gpsimd.dma_start(out=seg_t[:], in_=seg_d[:])
            nc.gpsimd.dma_start(out=recip_t[:], in_=recip_d[:])
            nc.gpsimd.dma_start(out=fixup_t[:], in_=fixup_d[:])
            nc.gpsimd.dma_start(out=wt_t[:], in_=wt_d[:])
            nc.gpsimd.dma_start(out=bias_t[:], in_=bias_d[:])



            prev = None  # (ps_a, ps_b, g-1)
            r_tile = None
            mgt = None

            for g in range(G + 1):
                cur = None
                if g < G:
                    x_t = xpool.tile(
                        [128, TILES_PER_GROUP * D], dt.float16, tag="x"
                    )
                    nc.sync.dma_start(
                        out=x_t[:].rearrange("p (t d) -> p t d", d=D),
                        in_=x_d[
                            g * ROWS_PER_GROUP : (g + 1) * ROWS_PER_GROUP, :
                        ].rearrange("(t p) d -> p t d", p=ROWS_PER_TILE),
                    )
                    ps_a = pspool.tile([128, D_SPLIT], dt.float32, tag="psa")
                    ps_b = pspool.tile([128, D - D_SPLIT], dt.float32, tag="psb")
                    for t in range(TILES_PER_GROUP):
                        a_t = apool.tile([128, 128], dt.float16, tag="a")
                        col = g * TILES_PER_GROUP + t
                        nc.vector.tensor_scalar(
                            out=a_t[:],
                            in0=iota_t[:],
                            scalar1=seg_t[:, col : col + 1],
                            scalar2=None,
                            op0=mybir.AluOpType.is_equal,
                        )
                        first = t == 0
                        nc.tensor.matmul(
                            ps_a[:],
                            a_t[:],
                            x_t[:, t * D : t * D + D_SPLIT],
                            start=first,
                            stop=False,
                        )
                        nc.tensor.matmul(
                            ps_b[:],
                            a_t[:],
                            x_t[:, t * D + D_SPLIT : (t + 1) * D],
                            start=first,
                            stop=False,
                        )
                    cur = (ps_a, ps_b)
                    if g >= 1:
                        # row 0 of this group's partial sums, for the fixup
                        # of group g-1 (bag split across the boundary)
                        r_tile = rpool.tile([1, D], dt.float16, tag="r")
                        nc.scalar.copy(r_tile[:, 0:D_SPLIT], ps_a[0:1, :])
                        nc.scalar.copy(r_tile[:, D_SPLIT:D], ps_b[0:1, :])

                if g >= 1:
                    pg = g - 1
                    pa, pb = prev
                    # fixup: S_{g-1}[L] += S_g[0]; zero one-hot for no-op.
                    # For g == G reuse the last r_tile (one-hot is zero).
                    fx = fixup_t[:, pg * 128 : (pg + 1) * 128]
                    nc.tensor.matmul(
                        pa[:], fx, r_tile[:, 0:D_SPLIT], start=False, stop=True
                    )
                    nc.tensor.matmul(
                        pb[:], fx, r_tile[:, D_SPLIT:D], start=False, stop=True
                    )
                    # means = psum * recip (per output column of group pg)
                    means = mpool.tile([128, D], dt.float16, tag="m")
                    nc.scalar.activation(
                        means[:, 0:D_SPLIT],
                        pa[:],
                        mybir.ActivationFunctionType.Copy,
                        scale=recip_t[:, pg : pg + 1],
                    )
                    nc.scalar.activation(
                        means[:, D_SPLIT:D],
                        pb[:],
                        mybir.ActivationFunctionType.Copy,
                        scale=recip_t[:, pg : pg + 1],
                    )
                    # wait: recip scaling must be per *partition* = bag row
                    # of the psum ([bag, D] layout) -> scalar1 is [128,1] ok

                    h = pg % 2
                    if h == 0:
                        mgt = tpool.tile([128, 6 * 256], dt.float16, tag="mgt")
                    for d in range(D_CHUNKS):
                        w = 128 if d < 5 else D_LAST
                        ps_t = ptpool.tile([128, 128], dt.float16, tag="pt")
                        nc.tensor.transpose(
                            ps_t[0:w, :],
                            means[:, d * 128 : d * 128 + w],
                            ident_t[:],
                        )
                        if d % 2 == 0:
                            nc.vector.tensor_copy(
                                mgt[0:w, d * 256 + h * 128 : d * 256 + h * 128 + 128],
                                ps_t[0:w, :],
                            )
                        else:
                            nc.scalar.copy(
                                mgt[0:w, d * 256 + h * 128 : d * 256 + h * 128 + 128],
                                ps_t[0:w, :],
                            )
                    if h == 1 or g == G:
                        q = pg // 2
                        pp = pppool.tile([128, 256], dt.float32, tag="pp")
                        for d in range(D_CHUNKS):
                            w = 128 if d < 5 else D_LAST
                            nc.tensor.matmul(
                                pp[:],
                                wt_t[0:w, d * 128 : (d + 1) * 128],
                                mgt[0:w, d * 256 : (d + 1) * 256],
                                start=(d == 0),
                                stop=(d == D_CHUNKS - 1),
                            )
                        out_sb = opool.tile([C, 256], dt.float32, tag="o")
                        nc.scalar.activation(
                            out_sb[:],
                            pp[0:C, :],
                            mybir.ActivationFunctionType.Identity,
                            bias=bias_t[:],
                        )
                        nc.gpsimd.dma_start(
                            out=out_d[:, q * 256 : (q + 1) * 256], in_=out_sb[:]
                        )
                prev = cur
    return nc


def prepare(x, scope, rel_weight, bias):
    """Build the SPMD program + per-core input maps. Returns a dict with
    everything needed to execute and assemble the output."""
    split_multi_waits = _apply_walrus_workarounds()

    x = np.asarray(x, dtype=np.float32)
    scope_np = np.asarray(scope)
    rel_weight = np.asarray(rel_weight, dtype=np.float32)
    bias = np.asarray(bias, dtype=np.float32)
    n_bags = scope_np.shape[0] - 1

    cores, G, n_pairs = _preprocess(x, scope_np)
    nc = _build_program(G, n_pairs, rel_weight, bias)
    split_multi_waits(nc)

    iota = np.tile(np.arange(128, dtype=np.float32), (128, 1))
    ident = np.eye(128, dtype=np.float16)
    wt = np.zeros((128, 768), dtype=np.float16)
    wpad = np.zeros((C, 768), dtype=np.float32)
    wpad[:, :D] = rel_weight
    for d in range(6):
        wt[:, d * 128 : d * 128 + C] = wpad[:, d * 128 : (d + 1) * 128].T
    bias_in = bias.reshape(C, 1).copy()

    in_maps = []
    for c in range(N_CORES):
        cd = cores[c]
        in_maps.append(
            {
                "x": cd["x"],
                "seg": cd["seg"],
                "recip": cd["recip"],
                "fixup": cd["fixup"].astype(np.float16),
                "iota": iota,
                "ident": ident,
                "wt": wt,
                "bias": bias_in,
            }
        )

    def assemble(results):
        logits_t = np.empty((C, n_bags), dtype=np.float32)
        for c in range(N_CORES):
            out = results[c]["out"]  # [C, n_pairs*256]
            cd = cores[c]
            base, s_col, e_col = cd["base"], cd["start_col"], cd["end_col"]
            for g in range(G):
                s, e = int(s_col[g]), int(e_col[g])
                if e < s:
                    continue
                col0 = 256 * (g // 2) + 128 * (g % 2)
                bag0 = int(base[g])
                logits_t[:, bag0 + s : bag0 + e + 1] = out[
                    :, col0 + s : col0 + e + 1
                ]
        return np.ascontiguousarray(logits_t.T)

    return dict(nc=nc, in_maps=in_maps, assemble=assemble, G=G, n_pairs=n_pairs)


def kernel(x, scope, rel_weight, bias):
    from concourse.bass_utils import run_bass_kernel_spmd

    p = prepare(x, scope, rel_weight, bias)
    res = run_bass_kernel_spmd(p["nc"], p["in_maps"], list(range(N_CORES)))
    return p["assemble"](res.results)
